# revision 49
# baseline (speedup 1.0000x reference)
"""AtomicConvolution Trainium2 kernel (8 NeuronCores, SPMD, no collectives).

Sharding: N-shard. Core r handles atoms [256r, 256r+256) for ALL 16 batches.
The X coordinate table (tiny) is replicated per core, so the neighbor gather
is core-local and the batch-norm moments over axis 0 (batch) are core-local
too (each core holds all 16 batches for its atoms). No cross-core traffic.

Per-core pipeline (the GPSIMD ap_gather at ~23-30ns/group-index is the
bottleneck; everything else is scheduled to hide under it; DMA-engine
gathers (InstDMAGatherAnt) are NOT usable -- the deployed firmware's
mlp/attnmlp Q7 libraries hard-crash this device, only the ap_gather
library is good):
  - gather table tbl[16g+c, beta*2048 + j] = plane c of X[2g+beta, j]
    (partition-group g owns batches {2g, 2g+1}; c=0,1,2 -> x,y,z; c=3 -> 0)
  - ap_gather (d=1, 7 chunks of 4096 idx/group + the last chunk as two
    2048-idx halves so only a 2-ns stage trails the final gather; 2-deep
    output ring) pulls all 16 channels per index -> x,y,z in one read;
    tbl/gidx are double-buffered so the next rep's loads overlap the tail
  - dx = gathered - centers (broadcast AP over m), squared in place;
    R^2 = PE ones-block matmul reducing the 4 channels; ACT Sqrt;
    one SBUF->SBUF DMA compacts R to [128, 2048] with partition p = 8b + nb,
    free = (ns, m), atom n = 256r + 32 nb + ns
  - rsf_l = exp(-re(R-rs)^2) * 0.5*(cos(pi R/rc)+1) * [R<=rc]:
      u = Square(R - rs)  (ACT); K' = Exp(-re*u + ln 0.5)  (ACT)
      cos(u') via Sin(pi/2 - Relu(pi - pi R/rc)): exact cutoff, clamped arg
      rsf = (cos - 1) * K' = -K'*FC  (negation absorbed in the BN subtract)
  - per l: ONE broadcast multiply against the packed 4-type mask + ONE
    segmented reduce into a transposed sym accumulator; the R path stays
    f32 (exp(-re(R-rs)^2) is brutally sensitive to R error) but the
    post-exp path (c1/kp/rsf/mask4/pm4) runs in fp16: halves quarter-stage
    SBUF traffic (less port contention vs the GPSIMD random reads) and
    enables 2x DVE modes; end-to-end rel err ~1.8e-3 vs the 2e-2 gate
  - BN over the 16 batches: PE stride-8 partition reductions + broadcasts;
    the final multiply writes through a strided AP to restore (ns, a*12+l)
    output order. One 4-ns rsf/BN stage per gather chunk (chunk k runs
    chunk k-1's stage), emitted BEFORE chunk k's DVE ops so the in-order
    DVE queue never head-of-line blocks on the gather; only chunk 7's
    stage trails the final gather.
Timing notes:
  - Measure with ADJACENT (reps=1, reps=R) pairs and take the median of
    paired differences (see test.py). The host/axon overhead level shifts
    by tens of ms mid-session; the naive min-over-blocks reps-difference
    then swings the apparent kernel time by -60%/+50% (one artifact read
    430us for a 1.07ms kernel).
  - With paired timing, ap_gather measures 32.0 ns/group-index on this
    device BOTH standalone and in-kernel (the "22.8ns standalone" floor in
    earlier notes was the artifact above). The full kernel at ~1.07ms/rep
    is ~98% gather-bound: bare gathers alone are ~1.05ms/rep, so only
    ~20us of loads/launch/tail overhead is not hidden. Further gains need
    a faster gather engine, which this firmware does not offer.
"""
import sys

if '/opt/trn_rl_repo' not in sys.path:
    sys.path.insert(0, '/opt/trn_rl_repo')

import math
import numpy as np

import concourse.bacc as bacc
import concourse.bass as bass
import concourse.mybir as mybir
from concourse import library_config
from concourse.tile import TileContext

# Steer the act-table-load pass: every ACT func this kernel uses (exp, ln,
# square, relu) lives in natural_log_exp_and_others, but the insertion pass
# maps each func to the FIRST set containing it, so ln<->exp would reload the
# table on every switch. Strip those funcs from all other sets (set order and
# ids are unchanged) so everything resolves to the one shared set and the
# single load hoists out of the loop.
import concourse.hw_specs as _hw_specs
if not getattr(_hw_specs, "_act_tbl_patched", False):
    _orig_gat = _hw_specs.get_activation_tables

    def _gat_one_set(arch):
        tabs = _orig_gat(arch)
        keep = "natural_log_exp_and_others"
        if keep not in tabs:
            return tabs
        shared = tabs[keep]
        return {name: (fs if name == keep else fs - shared)
                for name, fs in tabs.items()}

    _hw_specs.get_activation_tables = _gat_one_set
    _hw_specs._act_tbl_patched = True
    bacc.get_activation_tables = _gat_one_set

# Give the Tile scheduler realistic ap_gather timing. The stock cost model
# rates a 4096-idx gather at ~5.7us; measured hardware is ~93us. With the
# stock number the static schedule believes gathers finish instantly and
# orders the next chunk's DVE ops ahead of the previous R-stage, so the
# in-order DVE queue head-of-line blocks on the gather and ~120us of stage
# work drains serially after the last gather. 0.0366 calibrates the model
# to 22.8ns/group-idx. Must run before the first build in the process (the
# rust side caches the spec in a OnceLock).
if "APGather" not in _hw_specs.TRN2Spec.GPSIMD_IMPL_EFFICIENCY:
    for _k in ("APGather", "InstAPGather", "ISA"):
        _hw_specs.TRN2Spec.GPSIMD_IMPL_EFFICIENCY[_k] = 0.0366

F32 = mybir.dt.float32
F16 = mybir.dt.float16
BF16 = mybir.dt.bfloat16
I16 = mybir.dt.int16
AF = mybir.ActivationFunctionType
ALU = mybir.AluOpType

P = 128
B, N, M, L, A = 16, 2048, 64, 12, 4
NSH = N // 8                 # atoms per core = 256
NCHUNK = 8                   # gather chunks per core
CI = 4096                    # indices per group per chunk
TF = 2048                    # compacted R free size (= 32 ns * 64 m)
NFEAT = A * L                # 48
OUTF = 32 * NFEAT            # 1536 output cols per partition
ATOM_TYPES = (1, 6, 7, 8)
BN_EPS = 1e-3
PI = math.pi
GCH_BUFS = 2
# cos(x) on [0, pi] as a degree-5 polynomial in y = x^2 (max err 2.4e-6)
COS_B = (0.9999994437, -0.4999955817, 0.0416610328, -0.0013862747,
         2.42532e-05, -2.219e-07)


def build_nc(rc_v, rs_v, re_v, reps=None, ablate=()):
    """Build the per-core graph. rc/rs/re are baked in as immediates.
    reps: if set, wrap the whole body in a HW For_i loop (for benchmarking).
    ablate: subset of {"gather","prod","mm","quarter"} to skip (profiling)."""
    ablate = set(ablate)
    rc_v = [float(x) for x in rc_v]
    rs_v = [float(x) for x in rs_v]
    re_v = [float(x) for x in re_v]
    rc_groups = {}
    for l, v in enumerate(rc_v):
        rc_groups.setdefault(v, []).append(l)
    rc_list = list(rc_groups.keys())
    rcg_of_l = {}
    for gi, v in enumerate(rc_list):
        for l in rc_groups[v]:
            rcg_of_l[l] = gi

    nc = bacc.Bacc()
    tbl_in = nc.declare_dram_parameter("tbl", [P, 2 * N], F32, isOutput=False)
    gidx_in = nc.declare_dram_parameter("gidx", [P, TF], I16, isOutput=False)
    cen_in = nc.declare_dram_parameter("cen", [P, 2 * NSH], F32, isOutput=False)
    zc_in = nc.declare_dram_parameter("zc", [P, TF], F16, isOutput=False)
    wq_in = nc.declare_dram_parameter("wq", [P, 8], F32, isOutput=False)
    bnred_in = nc.declare_dram_parameter("bnred", [P, 8], F32, isOutput=False)
    bnbc_in = nc.declare_dram_parameter("bnbc", [8, P], F32, isOutput=False)
    cb_in = nc.declare_dram_parameter("cbias", [P, 32], F32, isOutput=False)
    out_ext = nc.declare_dram_parameter("out", [P, OUTF], F32, isOutput=True)

    rr_dram = nc.dram_tensor("rr", [NCHUNK, 8, CI], F32)

    import contextlib
    with TileContext(nc) as tc:
        with tc.tile_pool(name="sbuf", bufs=1) as pool, \
             tc.tile_pool(name="psum", bufs=1, space="PSUM") as psum:
            nc.gpsimd.load_library(library_config.ap_gather)
            loop_cm = tc.For_i(0, reps, 1) if reps else contextlib.nullcontext()
            _body_build(nc, tc, pool, psum, loop_cm,
                        tbl_in, gidx_in, cen_in, zc_in, wq_in,
                        bnred_in, bnbc_in, cb_in, out_ext, rr_dram,
                        rc_list, rcg_of_l, rs_v, re_v, ablate)
    nc.compile()
    return nc


def _body_build(nc, tc, pool, psum, loop_cm,
                tbl_in, gidx_in, cen_in, zc_in, wq_in,
                bnred_in, bnbc_in, cb_in, out_ext, rr_dram,
                rc_list, rcg_of_l, rs_v, re_v, ablate=()):
    pure = "pure" in ablate
    with loop_cm:
            # tbl/gidx double-buffered so the next rep's input loads overlap
            # this rep's tail compute instead of serializing at the loop edge
            tbl = pool.tile([P, 2 * N], F32, tag="tbl", bufs=2)
            gidx = pool.tile([P, TF], I16, tag="gidx", bufs=2)
            cen = pool.tile([P, 2 * NSH], F32)
            zc = pool.tile([P, TF], F16)
            wq = pool.tile([P, 8], F32)
            bnred = pool.tile([P, 8], F32)
            bnbc = pool.tile([8, P], F32)
            cb = pool.tile([P, 32], F32)
            # split loads across the two HWDGE queues (SP + ACT) to overlap
            for t, src in [(tbl, tbl_in), (gidx, gidx_in), (wq, wq_in),
                           (bnred, bnred_in)]:
                nc.sync.dma_start(out=t[:], in_=src[:])
            for t, src in [(zc, zc_in), (cen, cen_in),
                           (bnbc, bnbc_in), (cb, cb_in)]:
                nc.scalar.dma_start(out=t[:], in_=src[:])

            sym = pool.tile([P, OUTF], F32)
            if "pm" in ablate:
                nc.vector.memset(sym[:], 1.0)
            Rt = pool.tile([P, TF], F32)
            cen_pitch = cen[:].ap[0][0]

            # 7 full 4096-idx chunks + the last chunk split into two 2048-idx
            # halves (host packs its indices (j2, beta, nb, j1, m)) so the
            # second-to-last stage hides under the final half-gather and only
            # a 2-ns stage trails it
            chunks = [(256 * k, CI, 4 * k) for k in range(7)] + \
                     [(1792 + 64 * q, CI // 4, 28 + q) for q in range(4)]
            prev_stage = None
            for colo, ci, ns0 in chunks:
                jext = ci // 1024
                gch = pool.tile([P, CI], F32, tag="gch", bufs=GCH_BUFS)
                if "gather" not in ablate:
                    nc.gpsimd.ap_gather(
                        out_ap=gch[:, 0:ci], in_ap=tbl[:],
                        idxs_ap=gidx[:, colo:colo + ci // 16],
                        channels=P, num_elems=2 * N, d=1, num_idxs=ci)
                else:
                    nc.vector.memset(gch[:], 1.0)
                if pure:
                    continue

                # the previous chunk's rsf/BN stage runs while this gather is
                # in flight; emitted BEFORE this chunk's DVE ops so the
                # in-order DVE queue doesn't head-of-line block on gch
                if prev_stage is not None and "mm" not in ablate \
                        and "quarter" not in ablate:
                    _quarter(nc, pool, psum, Rt, zc, sym, bnred, bnbc, cb,
                             out_ext, prev_stage[0], prev_stage[1],
                             rc_list, rcg_of_l, rs_v, re_v, ablate)
                prev_stage = (64 * ns0, jext)

                # ---- dx = gathered - centers, then dx*dx (ch3 is 0-0=0)
                half = ci // 2
                dxt = pool.tile([P, CI], F32, tag="dxt", bufs=2)
                if "prod" in ablate:
                    nc.vector.memset(dxt[:], 1.0)
                for beta in range(2):
                    if "prod" in ablate:
                        break
                    cfree = ([[32, 8], [1, jext], [0, 64]] if jext > 1
                             else [[32, 8], [0, 64]])
                    cen_ap = bass.AP(
                        cen.tensor,
                        cen[:].offset + 256 * beta + ns0,
                        [[cen_pitch, P]] + cfree)
                    nc.vector.tensor_tensor(
                        out=dxt[:, half * beta:half * (beta + 1)],
                        in0=gch[:, half * beta:half * (beta + 1)],
                        in1=cen_ap, op=ALU.subtract)
                nc.vector.tensor_tensor(out=dxt[:, 0:ci], in0=dxt[:, 0:ci],
                                        in1=dxt[:, 0:ci], op=ALU.mult)

                # ---- R^2 via PE, then R = exp(0.5 ln R^2) (stays in the
                # ln/exp ACT set; a Sqrt would force a table reload)
                if "mm" in ablate:
                    continue
                rsp = pool.tile([8, CI], F32, tag="rsp", bufs=1)
                for h in range(ci // 1024):
                    ps = psum.tile([8, 1024], F32, tag="pchunk", bufs=2)
                    for j in range(2):
                        c0 = 1024 * h + 512 * j
                        nc.tensor.matmul(out=ps[:, 512 * j:512 * j + 512],
                                         lhsT=wq[:], rhs=dxt[:, c0:c0 + 512],
                                         start=True, stop=True)
                    hs = slice(1024 * h, 1024 * h + 1024)
                    nc.scalar.activation(out=rsp[0:8, hs], in_=ps[:],
                                         func=AF.Ln)
                    nc.scalar.activation(out=rsp[0:8, hs], in_=rsp[0:8, hs],
                                         func=AF.Exp, scale=0.5)
                # SBUF->SBUF compaction [8, (p f)] -> [(g p), f]
                nc.sync.dma_start(
                    out=Rt[:, 64 * ns0:64 * ns0 + ci // 16],
                    in_=rsp[0:8, 0:ci].rearrange("g (p f) -> g p f", p=16))

            if not pure and "mm" not in ablate and "quarter" not in ablate:
                _quarter(nc, pool, psum, Rt, zc, sym, bnred, bnbc, cb,
                         out_ext, prev_stage[0], prev_stage[1],
                         rc_list, rcg_of_l, rs_v, re_v, ablate)


def _quarter(nc, pool, psum, Rt, zc, sym, bnred, bnbc, cb, out_ext,
             c0, nsc, rc_list, rcg_of_l, rs_v, re_v, ablate=()):
    """rsf + masked reduce + BN for R columns [c0, c0 + 64*nsc).

    sym is the transposed accumulator [(stage, l, a, ns) blocks]; the final
    BN multiply writes through a strided AP to restore (ns, a, l) order.
    """
    W = 64 * nsc
    fsl = slice(c0, c0 + W)
    ns0 = c0 // 64

    c1s = []
    for gi, rcval in enumerate(rc_list):
        ur = pool.tile([P, 512], F32, tag="ur", bufs=1)
        nc.scalar.activation(out=ur[:, 0:W], in_=Rt[:, fsl], func=AF.Relu,
                             scale=-PI / rcval, bias=cb[:, 0:1])
        # cos(ur) via degree-5 polynomial in y = ur^2 (max err 2.4e-6 on
        # [0, pi]); Square/Relu live in every ACT function set, so unlike
        # Sin this costs no 1.3us table reload per use
        yy = pool.tile([P, 512], F32, tag="yy", bufs=1)
        nc.scalar.activation(out=yy[:, 0:W], in_=ur[:, 0:W], func=AF.Square)
        t = pool.tile([P, 512], F32, tag="ct", bufs=1)
        nc.vector.tensor_scalar(out=t[:, 0:W], in0=yy[:, 0:W],
                                scalar1=COS_B[5], scalar2=COS_B[4],
                                op0=ALU.mult, op1=ALU.add)
        for kk in (3, 2, 1):
            nc.vector.tensor_tensor(out=t[:, 0:W], in0=t[:, 0:W],
                                    in1=yy[:, 0:W], op=ALU.mult)
            nc.vector.tensor_scalar(out=t[:, 0:W], in0=t[:, 0:W],
                                    scalar1=COS_B[kk], scalar2=None,
                                    op0=ALU.add)
        nc.vector.tensor_tensor(out=t[:, 0:W], in0=t[:, 0:W],
                                in1=yy[:, 0:W], op=ALU.mult)
        c1 = pool.tile([P, 512], F16, tag=f"c1_{gi}")
        nc.vector.tensor_scalar(out=c1[:, 0:W], in0=t[:, 0:W],
                                scalar1=COS_B[0], scalar2=None, op0=ALU.add)
        c1s.append(c1)

    # 4 type masks packed [128, (a, i)] so each l needs ONE mask multiply
    mask4 = pool.tile([P, 4 * 512], F16, tag="mask4", bufs=1)
    for a in range(A):
        nc.vector.tensor_scalar(out=mask4[:, 512 * a:512 * a + W],
                                in0=zc[:, fsl],
                                scalar1=float(ATOM_TYPES[a]), scalar2=None,
                                op0=ALU.is_equal)

    # all 12 u's and kp's in wide tiles with no ring reuse: every per-l DVE
    # chain is dependency-ready the moment Rt lands, so the tile scheduler's
    # optimistic gather timing (v1 cost model has no GPSIMD efficiency) can
    # interleave next-chunk ops into the engine order without stalling this
    # stage behind the 93us gather
    u12 = pool.tile([P, 12 * 256], F16, tag="u12", bufs=1)
    kp12 = pool.tile([P, 12 * 256], F16, tag="kp12", bufs=1)
    for l in range(L):
        nc.scalar.activation(out=u12[:, 256 * l:256 * l + W], in_=Rt[:, fsl],
                             func=AF.Square, scale=1.0,
                             bias=cb[:, 16 + l:17 + l])
    for l in range(L):
        nc.scalar.activation(out=kp12[:, 256 * l:256 * l + W],
                             in_=u12[:, 256 * l:256 * l + W], func=AF.Exp,
                             scale=-re_v[l], bias=cb[:, 3:4])
    for l in range(L):
        rsf = pool.tile([P, 512], F16, tag="rsf", bufs=2)
        nc.vector.scalar_tensor_tensor(
            out=rsf[:, 0:W], in0=c1s[rcg_of_l[l]][:, 0:W], scalar=1.0,
            in1=kp12[:, 256 * l:256 * l + W], op0=ALU.subtract,
            op1=ALU.mult)  # -K'*FCx2
        if "pm" in ablate:
            continue
        # one multiply for all 4 type masks: rsf broadcast over the a axis
        pm4 = pool.tile([P, 4 * 512], F16, tag="pm4", bufs=1)
        rsf_b = bass.AP(rsf.tensor, rsf[:].offset,
                        [[rsf[:].ap[0][0], P], [0, 4], [1, W]])
        pm4_w = bass.AP(pm4.tensor, pm4[:].offset,
                        [[pm4[:].ap[0][0], P], [512, 4], [1, W]])
        nc.vector.tensor_tensor(out=pm4_w, in0=rsf_b, in1=bass.AP(
            mask4.tensor, mask4[:].offset,
            [[mask4[:].ap[0][0], P], [512, 4], [1, W]]), op=ALU.mult)
        # one segmented reduce -> contiguous [128, (a, ns)] block of sym
        base = 48 * ns0 + l * 4 * nsc
        pm4_r = bass.AP(pm4.tensor, pm4[:].offset,
                        [[pm4[:].ap[0][0], P], [512, 4], [64, nsc], [1, 64]])
        nc.vector.tensor_reduce(
            out=sym[:, base:base + 4 * nsc], in_=pm4_r,
            axis=mybir.AxisListType.X, op=ALU.add)

    # ---- batch-norm for this stage's 48*nsc sym cols [(l, a, ns) layout]
    CW = 48 * nsc
    cf = slice(48 * ns0, 48 * ns0 + CW)
    if "bn" in ablate:
        if "pm" not in ablate:
            nc.sync.dma_start(out=out_ext[:, cf], in_=sym[:, cf])
        return
    ssq = pool.tile([P, 384], F32, tag="ssq", bufs=1)
    nc.vector.tensor_tensor(out=ssq[:, 0:CW], in0=sym[:, cf], in1=sym[:, cf],
                            op=ALU.mult)
    pm1 = psum.tile([8, 384], F32, tag="pbn0")
    nc.tensor.matmul(out=pm1[:, 0:CW], lhsT=bnred[:], rhs=sym[:, cf],
                     start=True, stop=True)
    pm2 = psum.tile([8, 384], F32, tag="pbn1")
    nc.tensor.matmul(out=pm2[:, 0:CW], lhsT=bnred[:], rhs=ssq[:, 0:CW],
                     start=True, stop=True)
    msb = pool.tile([8, 384], F32, tag="msb", bufs=1)
    nc.vector.tensor_copy(out=msb[0:8, 0:CW], in_=pm1[:, 0:CW])
    m2 = pool.tile([8, 384], F32, tag="m2", bufs=1)
    nc.vector.tensor_tensor(out=m2[0:8, 0:CW], in0=msb[0:8, 0:CW],
                            in1=msb[0:8, 0:CW], op=ALU.mult)
    vsb = pool.tile([8, 384], F32, tag="vsb", bufs=1)
    nc.vector.tensor_tensor(out=vsb[0:8, 0:CW], in0=pm2[:, 0:CW],
                            in1=m2[0:8, 0:CW], op=ALU.subtract)
    # 1/sqrt(v + eps) = exp(-0.5 ln(v + eps)): stays in the ln/exp ACT set
    # and drops the DVE reciprocal
    ssb = pool.tile([8, 384], F32, tag="ssb", bufs=1)
    nc.scalar.activation(out=ssb[0:8, 0:CW], in_=vsb[0:8, 0:CW], func=AF.Ln,
                         bias=cb[0:8, 2:3])
    rsb = pool.tile([8, 384], F32, tag="rsb", bufs=1)
    nc.scalar.activation(out=rsb[0:8, 0:CW], in_=ssb[0:8, 0:CW], func=AF.Exp,
                         scale=-0.5)
    pbm = psum.tile([P, 384], F32, tag="pbn2")
    nc.tensor.matmul(out=pbm[:, 0:CW], lhsT=bnbc[:], rhs=msb[0:8, 0:CW],
                     start=True, stop=True)
    pbr = psum.tile([P, 384], F32, tag="pbn3")
    nc.tensor.matmul(out=pbr[:, 0:CW], lhsT=bnbc[:], rhs=rsb[0:8, 0:CW],
                     start=True, stop=True)
    dsb = pool.tile([P, 384], F32, tag="dsb", bufs=1)
    nc.vector.tensor_tensor(out=dsb[:, 0:CW], in0=pbm[:, 0:CW], in1=sym[:, cf],
                            op=ALU.subtract)
    # final multiply writes transposed: (l, a, ns) walk -> col ns*48 + a*12 + l
    osb = pool.tile([P, 384], F32, tag="osb", bufs=2)
    dsb_v = dsb[:, 0:CW].rearrange("p (l a s) -> p l a s", l=12, a=4)
    pbr_v = pbr[:, 0:CW].rearrange("p (l a s) -> p l a s", l=12, a=4)
    osb_w = bass.AP(osb.tensor, osb[:].offset,
                    [[osb[:].ap[0][0], P], [1, 12], [12, 4], [48, nsc]])
    nc.vector.tensor_tensor(out=osb_w, in0=dsb_v, in1=pbr_v, op=ALU.mult)
    nc.sync.dma_start(out=out_ext[:, cf], in_=osb[:, 0:CW])


# ---------------------------------------------------------------- host side

def make_cbias(rs_v, re_v):
    cb = np.zeros((P, 32), np.float32)
    cb[:, 0] = PI
    cb[:, 1] = 0.5 * PI
    cb[:, 2] = BN_EPS
    cb[:, 3] = math.log(0.5)
    for l in range(L):
        cb[:, 16 + l] = -float(rs_v[l])
    return cb


def prep_core_inputs(X, Nbrs, Nbrs_Z, r, const_cache={}):
    """Build core r's input map (numpy layout prep only)."""
    n0 = NSH * r
    Xt = np.ascontiguousarray(X.transpose(2, 0, 1))          # [3, B, N]
    if "tbl" not in const_cache:
        tbl = np.zeros((8, 16, 2, N), np.float32)
        tbl[:, 0:3, :, :] = Xt.reshape(3, 8, 2, N).transpose(1, 0, 2, 3)
        const_cache["tbl"] = tbl.reshape(P, 2 * N)

        wq = np.zeros((P, 8), np.float32)
        for g in range(8):
            wq[16 * g + 0:16 * g + 3, g] = 1.0
        bnred = np.zeros((P, 8), np.float32)
        bnbc = np.zeros((8, P), np.float32)
        for p in range(P):
            bnred[p, p % 8] = 1.0 / 16.0
            bnbc[p % 8, p] = 1.0
        const_cache["wq"] = wq
        const_cache["bnred"] = bnred
        const_cache["bnbc"] = bnbc
        const_cache["cbias"] = None  # filled by caller

    cen = np.zeros((8, 16, 2, NSH), np.float32)
    cen[:, 0:3, :, :] = (Xt[:, :, n0:n0 + NSH]
                         .reshape(3, 8, 2, NSH).transpose(1, 0, 2, 3))
    cen = cen.reshape(P, 2 * NSH)

    nbr_sh = Nbrs[:, n0:n0 + NSH, :]                          # [16, 256, 64]
    nbr6 = nbr_sh.reshape(8, 2, 8, 8, 4, M)                   # [g, beta, nb, k, j, m]
    lg = nbr6 + (np.arange(2, dtype=nbr6.dtype)
                 .reshape(1, 2, 1, 1, 1, 1) * N)
    lgt = lg.transpose(0, 3, 1, 2, 4, 5)                      # [g, k, beta, nb, j, m]
    main = lgt[:, :7].reshape(8, 7 * CI)
    # last chunk split into four 1024-idx quarters: (j, beta, nb, m) so each
    # quarter covers both betas and exactly one ns
    last = (lgt[:, 7]                                         # [g, beta, nb, j, m]
            .transpose(0, 3, 1, 2, 4).reshape(8, CI))
    lg = np.concatenate([main, last], axis=1)
    gidx = (lg.reshape(8, TF, 16).transpose(0, 2, 1)
            .reshape(P, TF).astype(np.int16))

    zc = (Nbrs_Z[:, n0:n0 + NSH, :].reshape(8, 2, 8, 32, M)
          .reshape(P, TF).astype(np.float16))

    return {"tbl": const_cache["tbl"], "gidx": gidx, "cen": cen, "zc": zc,
            "wq": const_cache["wq"], "bnred": const_cache["bnred"],
            "bnbc": const_cache["bnbc"], "cbias": const_cache["cbias"]}


def assemble_output(results):
    full = np.empty((8, 2, N, NFEAT), np.float32)             # [g, beta, n, f]
    for r in range(8):
        o = np.asarray(results[r]["out"]).reshape(8, 2, NSH, NFEAT)
        n0 = NSH * r
        full[:, :, n0:n0 + NSH, :] = o
    return full.reshape(B, N, NFEAT)


_cache = {}


def kernel(X, Nbrs, Nbrs_Z, rc, rs, re):
    from concourse.bass_utils import run_bass_kernel_spmd
    key = (tuple(np.asarray(rc).ravel().tolist()),
           tuple(np.asarray(rs).ravel().tolist()),
           tuple(np.asarray(re).ravel().tolist()))
    if key not in _cache:
        _cache[key] = build_nc(np.asarray(rc).ravel(), np.asarray(rs).ravel(),
                               np.asarray(re).ravel())
    nc = _cache[key]
    X = np.asarray(X, np.float32)
    Nbrs = np.asarray(Nbrs)
    Nbrs_Z = np.asarray(Nbrs_Z)
    cc = {}
    in_maps = [prep_core_inputs(X, Nbrs, Nbrs_Z, r, cc) for r in range(8)]
    cbias = make_cbias(np.asarray(rs).ravel(), np.asarray(re).ravel())
    for im in in_maps:
        im["cbias"] = cbias
    res = run_bass_kernel_spmd(nc, in_maps, core_ids=list(range(8)))
    return assemble_output(res.results)

